# revision 7
# baseline (speedup 1.0000x reference)
"""DSSIM loss on 8 Trainium2 NeuronCores (Bass/Tile).

Strategy (pure data parallel over batch, 4 images/core, 12 channel-images/core):
  signals u = x+y, v = x-y, u^2, v^2 are blurred with the separable 11x11
  kernel via two windowed "X-stationary" matmul passes:
      matmul(psum, lhsT=data_block[K=128,M=128], rhs=Toeplitz_band[K=128,N<=138])
  contracts the partition axis and emits the conv TRANSPOSED, so no explicit
  transpose is ever materialized; PSUM has_written accumulation handles the
  band-window overlaps and zero padding (clamped windows).
  Pass2 accumulates PE-side linear combos:  p=blur2(u), q=blur2(v),
  c=(blur2(u^2)-blur2(v^2))/2 = 2*blur2(xy),  e=(blur2(u^2)+blur2(v^2))/2
  = blur2(x^2+y^2).  Then per-pixel:
      a = (p^2-q^2)/2 = 2*mu1*mu2        b = (p^2+q^2)/2 = mu1^2+mu2^2
      num = (a+C1)(c-a+C2)               den = (b+C1)(e-b+C2)
      dssim_mean = 0.5 - 0.5*mean(num/den)

Engine balancing (v2):
  - inputs are cast fp32->fp16 in the DMA itself (gpsimd SWDGE cast), so
    u,v tensor ops run in DVE 2x mode.
  - pass1 PSUM evacuation is split ACT/Pool (was: all ACT, the bottleneck).
  - ratio+mean fused in ONE custom DVE op (bitwise-not reciprocal seed +
    1 Newton step + multiply + accumulate), replacing reciprocal + TTR.
  - conv windows emit one matmul per contraction block (no accumulate/fresh
    segment splitting; per-element PSUM has_written handles mixed regions
    on hardware).  MERGED=False restores split segments for CoreSim.
"""

import math
import os

import numpy as np

import concourse.bass as bass
import concourse.bacc as bacc
import concourse.mybir as mybir
import concourse.tile as tile
import concourse.dve_ops as dve_ops
from concourse.dve_uop import DveOpSpec
from contextlib import ExitStack

F = 11
PAD = 5
C1 = 0.01 ** 2
C2 = 0.03 ** 2
H = 512
W = 512
NCORES = 8
BATCH = 32
CHAN = 3
B_PER_CORE = BATCH // NCORES          # 4
NCH = B_PER_CORE * CHAN               # 12 channel-images per core
NPOS = 4                              # 128-row blocks per image
TBW = 143                             # Toeplitz band tensor width

PSUM_DT = mybir.dt.float32   # matmul outputs must be fp32 (bass assert)
SBUF_DT = mybir.dt.float16
MERGED = os.environ.get("BASS_SSIM_MERGED", "1") == "1"   # 0 for CoreSim
DMA_CAST = os.environ.get("BASS_SSIM_DMACAST", "1") == "1"

f32 = mybir.dt.float32
f16 = mybir.dt.float16


# ---------------------------------------------------------------------------
# custom fused DVE ops
# ---------------------------------------------------------------------------
def _register_op(name, spec_fn):
    from concourse.dve_spec import lower, _has_src1 as has_src1

    op = dve_ops.DveOp(name, spec_fn(), subdim=False, uops_sha={})
    for ver in ("v3", "v4"):
        try:
            spec_obj = DveOpSpec(
                name=op.name, opcode=0, uops=lower(op.spec, ver=ver),
                rd1_en=has_src1(op.spec),
            )
            op.uops_sha[ver] = spec_obj.sha(ver)
        except Exception:
            pass
    if op.name not in dve_ops._SUB_OPCODE_FOR_NAME:
        dve_ops.OPS.append(op)
        dve_ops.CUSTOM_DVE_SPECS[op.name] = op.spec
        row = dve_ops._CUSTOM_DVE_ROW_BASE + len(dve_ops.OPS) - 1
        assert row < 0x20
        dve_ops._SUB_OPCODE_FOR_NAME[op.name] = row
    return op


def _numden_spec():
    # out = (Src0 + C0) * ((Src1 - Src0) + C1)
    from concourse.dve_spec import Spec, Src0, Src1, C0, C1 as SC1

    ref = lambda in0, in1, s0, s1, imm2: (
        (in0.astype(np.float32) + np.float32(s0))
        * ((in1.astype(np.float32) - in0.astype(np.float32)) + np.float32(s1))
    ).astype(np.float32)
    return Spec(body=(Src0 + C0) * ((Src1 - Src0) + SC1), reference=ref)


# seed scale / NR constant for the bitwise-not reciprocal approximation
# (same Chebyshev pair as RECIPROCAL_APPROX_FAST, one NR step: ~0.4% max err,
#  mean-of-ratios error far below the 2e-2 gate).
_RCP_S0 = -0.23549792
_RCP_S1 = 2.0017324


def _divacc_spec():
    # out = Src0 * recip1(Src1);  accum_out = sum(out)
    # recip1(d) = y0*(C1 - d*y0), y0 = bitcast(~d)*C0
    from concourse.dve_spec import (
        Spec, Src0, Src1, C0, C1 as SC1, Bin, AluOp, Zero)

    def ref(in0, in1, s0, s1, imm2):
        d = in1.astype(np.float32)
        not_d = (~d.view(np.int32)).view(np.float32)
        y0 = not_d * np.float32(s0)
        y1 = y0 * (np.float32(s1) - d * y0)
        out = (in0.astype(np.float32) * y1).astype(np.float32)
        return out, out.reshape(out.shape[0], -1).sum(axis=-1, keepdims=True)

    y0 = Bin(AluOp.BITWISE_NOT, Src1, Src1) * C0
    y1 = y0 * (SC1 - Src1 * y0)
    return Spec(body=Src0 * y1, accum=AluOp.ADD, accum_init=Zero,
                reference=ref)


NUMOP = _register_op("SSIM_NUMDEN_ANT", _numden_spec)
DIVACC = _register_op("SSIM_DIVACC_ANT", _divacc_spec)


# ---------------------------------------------------------------------------
# host-side kernel factorization
# ---------------------------------------------------------------------------
def _round_moment(k, nmom=2):
    """Round 1-D kernel factor to fp16 preserving moments (greedy)."""
    k = np.asarray(k, np.float64)
    n = len(k)
    kd = k.astype(np.float16).astype(np.float64)
    lo = np.minimum(np.nextafter(k.astype(np.float16), -np.inf).astype(np.float64), kd)
    hi = np.maximum(np.nextafter(k.astype(np.float16), np.inf).astype(np.float64), kd)
    cand = [np.array([lo[i], hi[i]]) for i in range(n)]
    idx = np.arange(n, dtype=np.float64) - (n - 1) / 2
    moms = np.stack([idx ** m for m in range(nmom + 1)])
    targ = moms @ k
    scale = np.abs(moms) @ np.abs(k) + 1e-300
    best = np.array([c[np.argmin(np.abs(c - k[i]))] for i, c in enumerate(cand)])

    def cost(v):
        return np.sum(((moms @ v - targ) / scale) ** 2)

    cur = cost(best)
    for _ in range(100):
        improved = False
        for i in range(n):
            for c in cand[i]:
                if c != best[i]:
                    t = best.copy()
                    t[i] = c
                    ct = cost(t)
                    if ct < cur - 1e-30:
                        best, cur = t, ct
                        improved = True
        if not improved:
            break
    return best


def _factor_channel_kernels(gauss_kernel):
    """[3,1,11,11] -> per-channel fp16 (kc, kr) rank-1 factors."""
    kcs, krs = [], []
    for ch in range(CHAN):
        k2d = np.asarray(gauss_kernel[ch, 0], np.float64)
        U, s, Vt = np.linalg.svd(k2d)
        kc = U[:, 0] * math.sqrt(s[0])
        kr = Vt[0] * math.sqrt(s[0])
        if kc.sum() < 0:
            kc, kr = -kc, -kr
        rec = np.abs(np.outer(kc, kr) - k2d).max()
        assert rec <= 1e-5 * max(1e-30, np.abs(k2d).max()), (
            f"gauss_kernel channel {ch} is not rank-1 (err {rec}); "
            "this kernel only supports separable filters"
        )
        kcs.append(_round_moment(kc))
        krs.append(_round_moment(kr))
    return kcs, krs


def _make_tfull(k1d):
    """T[k, j] = k1d[j - k] for j-k in [0,11); [128, TBW] float16."""
    T = np.zeros((128, TBW), np.float64)
    for d in range(F):
        j = np.arange(128) + d
        valid = j < TBW
        T[np.arange(128)[valid], j[valid]] = k1d[d]
    return T.astype(np.float16)


def _window(kb):
    lo = max(0, 128 * kb - PAD)
    hi = min(512, 128 * kb + 128 + PAD)
    return lo, hi, lo - 128 * kb + PAD


# ---------------------------------------------------------------------------
# device program
# ---------------------------------------------------------------------------
def build_program(nc: bass.Bass):
    im1 = nc.dram_tensor("im1", [B_PER_CORE, CHAN, H, W], f32, kind="ExternalInput").ap()
    im2 = nc.dram_tensor("im2", [B_PER_CORE, CHAN, H, W], f32, kind="ExternalInput").ap()
    # Toeplitz band tensors, [128, CHAN*TBW]: pass1 (kc), pass2 (kr), +-kr/2
    tb1 = nc.dram_tensor("tb1", [128, CHAN * TBW], f16, kind="ExternalInput").ap()
    tb2 = nc.dram_tensor("tb2", [128, CHAN * TBW], f16, kind="ExternalInput").ap()
    tb2h = nc.dram_tensor("tb2h", [128, CHAN * TBW], f16, kind="ExternalInput").ap()
    tb2hn = nc.dram_tensor("tb2hn", [128, CHAN * TBW], f16, kind="ExternalInput").ap()
    out = nc.dram_tensor("acc", [128, NCH], f32, kind="ExternalOutput").ap()

    with tile.TileContext(nc) as tc:
        with ExitStack() as ctx:
            _build_tile(ctx, tc, im1, im2, (tb1, tb2, tb2h, tb2hn), out)
    nc.compile()
    return nc


def _build_tile(ctx, tc, im1, im2, tbs, out):
    nc = tc.nc
    tb_pool = ctx.enter_context(tc.tile_pool(name="tb", bufs=1))
    in_pool = ctx.enter_context(tc.tile_pool(name="inp", bufs=2))
    sig_pool = ctx.enter_context(tc.tile_pool(name="sig", bufs=2))
    zt_pool = ctx.enter_context(tc.tile_pool(name="zt", bufs=2))
    p1_pool = ctx.enter_context(tc.tile_pool(name="p1", bufs=2, space="PSUM"))
    p2_pool = ctx.enter_context(tc.tile_pool(name="p2", bufs=4, space="PSUM"))
    fld_pool = ctx.enter_context(tc.tile_pool(name="fld", bufs=2))
    mth_pool = ctx.enter_context(tc.tile_pool(name="mth", bufs=2))
    acc_pool = ctx.enter_context(tc.tile_pool(name="accp", bufs=1))

    # band tensors in SBUF: [128, CHAN*TBW] each
    tbt = []
    for name, t in zip(("tb1", "tb2", "tb2h", "tb2hn"), tbs):
        st = tb_pool.tile([128, CHAN * TBW], f16, name=f"s_{name}")
        nc.sync.dma_start(st[:], t[:])
        tbt.append(st)
    stb1, stb2, stb2h, stb2hn = tbt

    acc = acc_pool.tile([128, NCH], f32, name="acc_sbuf")

    def conv_group(ps, ps_off, srcs, ch):
        """Windowed banded matmuls accumulating into ps[:, ps_off:ps_off+512].

        srcs: list of (stb_tile, lh_col_fn) pairs; lh_col_fn(kb) gives the
        lhsT AP for contraction block kb.  MERGED emits one matmul per
        window (mixed accumulate/fresh regions resolved by per-element
        PSUM has_written on HW); otherwise split into uniform segments
        for CoreSim.
        """
        plan = []
        for si, (stb, lh_fn) in enumerate(srcs):
            prev_hi = None
            for kb in range(4):
                lo, hi, off = _window(kb)
                if MERGED or si > 0 or prev_hi is None:
                    segs = [(lo, hi)]
                else:
                    segs = [(lo, prev_hi), (prev_hi, hi)]
                for s0, s1 in segs:
                    plan.append((lh_fn, kb, stb, s0, s1, off + (s0 - lo)))
                prev_hi = hi
        for i, (lh_fn, kb, stb, s0, s1, o) in enumerate(plan):
            nc.tensor.matmul(
                ps[:, ps_off + s0: ps_off + s1],
                lhsT=lh_fn(kb),
                rhs=stb[:, ch * TBW + o: ch * TBW + o + (s1 - s0)],
                start=(i == 0),
                stop=(i == len(plan) - 1),
            )

    for ci in range(NCH):
        img, ch = divmod(ci, CHAN)

        # ---- load x, y as fp16 [128, 4*512] (4 row-blocks side by side);
        #      the SWDGE (gpsimd) DMA casts fp32->fp16 in flight ----
        if DMA_CAST:
            x = in_pool.tile([128, 4 * W], f16, tag="x")
            y = in_pool.tile([128, 4 * W], f16, tag="y")
            nc.gpsimd.dma_start(x[:].rearrange("p (a w) -> p a w", a=4),
                                im1[img, ch].rearrange("(a p) w -> p a w", p=128))
            nc.gpsimd.dma_start(y[:].rearrange("p (a w) -> p a w", a=4),
                                im2[img, ch].rearrange("(a p) w -> p a w", p=128))
        else:
            x = in_pool.tile([128, 4 * W], f32, tag="x")
            y = in_pool.tile([128, 4 * W], f32, tag="y")
            nc.sync.dma_start(x[:].rearrange("p (a w) -> p a w", a=4),
                              im1[img, ch].rearrange("(a p) w -> p a w", p=128))
            nc.sync.dma_start(y[:].rearrange("p (a w) -> p a w", a=4),
                              im2[img, ch].rearrange("(a p) w -> p a w", p=128))

        # ---- u, v on Pool (SBUF-only TT; Pool cannot touch PSUM),
        #      u2, v2 on DVE (fp16 2x) ----
        u = sig_pool.tile([128, 4 * W], SBUF_DT, tag="u")
        v = sig_pool.tile([128, 4 * W], SBUF_DT, tag="v")
        nc.gpsimd.tensor_add(u[:], x[:], y[:])
        nc.gpsimd.tensor_sub(v[:], x[:], y[:])
        u2 = sig_pool.tile([128, 4 * W], SBUF_DT, tag="u2")
        v2 = sig_pool.tile([128, 4 * W], SBUF_DT, tag="v2")
        nc.vector.tensor_tensor(u2[:], u[:], u[:], mybir.AluOpType.mult)
        nc.vector.tensor_tensor(v2[:], v[:], v[:], mybir.AluOpType.mult)

        # ---- pass 1: ZT_g[c, o] = sum_r g[r, c] * kc[o - r + 5] ----
        # Two column-blocks share one [128,1024] PSUM tile (one accumulation
        # group, halves the evacuation instruction count).  Evacuation is
        # ACT-heavy with one copy on DVE for balance.
        zts = []
        for gi, g in enumerate((u, v, u2, v2)):
            zt = zt_pool.tile([128, 4 * W], SBUF_DT, tag=f"zt{gi}")
            for cbp in range(2):
                ps = p1_pool.tile([128, 2 * W], PSUM_DT, tag="p1")
                srcs = []
                for half in range(2):
                    cb = 2 * cbp + half
                    lh = (lambda g_, cb_: lambda kb: g_[
                        :, kb * W + cb_ * 128: kb * W + cb_ * 128 + 128])(g, cb)
                    srcs.append((half * W, stb1, lh))
                plan = []
                for ps_off, stb, lh_fn in srcs:
                    prev_hi = None
                    for kb in range(4):
                        lo, hi, off = _window(kb)
                        if MERGED or prev_hi is None:
                            segs = [(lo, hi)]
                        else:
                            segs = [(lo, prev_hi), (prev_hi, hi)]
                        for s0, s1 in segs:
                            plan.append([ps_off, lh_fn, kb, stb, s0, s1,
                                         off + (s0 - lo), prev_hi is None,
                                         False])
                        prev_hi = hi
                    plan[-1][-1] = True  # stop at each half's last matmul
                for ps_off, lh_fn, kb, stb, s0, s1, o, first, last in plan:
                    nc.tensor.matmul(
                        ps[:, ps_off + s0: ps_off + s1],
                        lhsT=lh_fn(kb),
                        rhs=stb[:, ch * TBW + o: ch * TBW + o + (s1 - s0)],
                        start=first,
                        stop=last,
                    )
                dst = zt[:, cbp * 2 * W:(cbp + 1) * 2 * W]
                if gi == 3 and cbp == 1:
                    nc.vector.tensor_copy(dst, ps[:])
                else:
                    nc.scalar.activation(
                        dst, ps[:], mybir.ActivationFunctionType.Copy)
            zts.append(zt)
        ztu, ztv, ztu2, ztv2 = zts

        # ---- pass 2 + math, per position ----
        FIELD_SRC = {
            "p": ((ztu, stb2),),
            "q": ((ztv, stb2),),
            "c": ((ztu2, stb2h), (ztv2, stb2hn)),
            "e": ((ztu2, stb2h), (ztv2, stb2h)),
        }
        num = mth_pool.tile([128, 4 * W], f32, tag="num")
        den = mth_pool.tile([128, 4 * W], f32, tag="den")
        for m in range(NPOS):
            fld = {}
            for fname, srcs in FIELD_SRC.items():
                ps = p2_pool.tile([128, W], PSUM_DT, tag="p2")
                pairs = []
                for zt, stb in srcs:
                    lh = (lambda zt_, m_: lambda cb: zt_[
                        :, cb * W + m_ * 128: cb * W + m_ * 128 + 128])(zt, m)
                    pairs.append((stb, lh))
                conv_group(ps, 0, pairs, ch)
                fld[fname] = ps
            # squares with folded /2 (scale 1/sqrt2), evacuating p,q PSUM
            P2 = fld_pool.tile([128, W], SBUF_DT, tag="P2")
            Q2 = fld_pool.tile([128, W], SBUF_DT, tag="Q2")
            isq2 = 1.0 / math.sqrt(2.0)
            nc.scalar.activation(P2[:], fld["p"][:],
                                 mybir.ActivationFunctionType.Square, scale=isq2)
            nc.scalar.activation(Q2[:], fld["q"][:],
                                 mybir.ActivationFunctionType.Square, scale=isq2)
            a = mth_pool.tile([128, W], SBUF_DT, tag="a")
            b = mth_pool.tile([128, W], SBUF_DT, tag="b")
            nc.vector.tensor_sub(a[:], P2[:], Q2[:])
            nc.vector.tensor_add(b[:], P2[:], Q2[:])
            nc.vector._custom_dve(NUMOP, out=num[:, m * W:(m + 1) * W],
                                  in0=a[:], in1=fld["c"][:], s0=C1, s1=C2)
            nc.vector._custom_dve(NUMOP, out=den[:, m * W:(m + 1) * W],
                                  in0=b[:], in1=fld["e"][:], s0=C1, s1=C2)
        # fused ratio + reduce over the whole channel-image:
        # acc[ci] = sum(num * recip1(den))
        scr = mth_pool.tile([128, 4 * W], f32, tag="scr")
        nc.vector._custom_dve(
            DIVACC, out=scr[:], in0=num[:], in1=den[:],
            s0=_RCP_S0, s1=_RCP_S1,
            accum_out=acc[:, ci:ci + 1],
        )

    nc.sync.dma_start(out[:], acc[:])


# ---------------------------------------------------------------------------
# host entry point
# ---------------------------------------------------------------------------
_PROGRAM_CACHE = {}


def _get_program():
    if "nc" not in _PROGRAM_CACHE:
        nc = bacc.Bacc("TRN2", target_bir_lowering=False, debug=False,
                       num_devices=NCORES)
        _PROGRAM_CACHE["nc"] = build_program(nc)
    return _PROGRAM_CACHE["nc"]


def _host_tensors(gauss_kernel):
    kcs, krs = _factor_channel_kernels(np.asarray(gauss_kernel, np.float32))

    def pack(k1ds):
        # [128, CHAN*TBW]
        return np.concatenate([_make_tfull(k) for k in k1ds], axis=1)

    tb1 = pack(kcs)
    tb2 = pack(krs)
    tb2h = pack([0.5 * np.asarray(k, np.float64) for k in krs])
    tb2hn = pack([-0.5 * np.asarray(k, np.float64) for k in krs])
    return tb1, tb2, tb2h, tb2hn


def make_in_maps(image1, image2, gauss_kernel):
    image1 = np.asarray(image1, np.float32)
    image2 = np.asarray(image2, np.float32)
    tb1, tb2, tb2h, tb2hn = _host_tensors(gauss_kernel)
    in_maps = []
    for c in range(NCORES):
        s = slice(c * B_PER_CORE, (c + 1) * B_PER_CORE)
        in_maps.append({
            "im1": np.ascontiguousarray(image1[s]),
            "im2": np.ascontiguousarray(image2[s]),
            "tb1": tb1, "tb2": tb2, "tb2h": tb2h, "tb2hn": tb2hn,
        })
    return in_maps


def finish(acc_list):
    total = sum(np.asarray(a, np.float64).sum() for a in acc_list)
    n = float(BATCH * CHAN * H * W)
    return np.float32(0.5 - 0.5 * total / n)


def kernel(image1, image2, gauss_kernel):
    from concourse.bass_utils import run_bass_kernel_spmd

    nc = _get_program()
    in_maps = make_in_maps(image1, image2, gauss_kernel)
    res = run_bass_kernel_spmd(nc, in_maps, core_ids=list(range(NCORES)))
    return finish([r["acc"] for r in res.results])


# revision 9
# speedup vs baseline: 1.1219x; 1.1219x over previous
"""DSSIM loss on 8 Trainium2 NeuronCores (Bass/Tile).

Strategy (pure data parallel over batch, 4 images/core, 12 channel-images/core):
  signals u = x+y, v = x-y, u^2, v^2 are blurred with the separable 11x11
  kernel via two windowed "X-stationary" matmul passes:
      matmul(psum, lhsT=data_block[K=128,M=128], rhs=Toeplitz_band[K=128,N<=138])
  contracts the partition axis and emits the conv TRANSPOSED, so no explicit
  transpose is ever materialized; PSUM has_written accumulation handles the
  band-window overlaps and zero padding (clamped windows).
  Pass2 accumulates PE-side linear combos:  p=blur2(u), q=blur2(v),
  c=(blur2(u^2)-blur2(v^2))/2 = 2*blur2(xy),  e=(blur2(u^2)+blur2(v^2))/2
  = blur2(x^2+y^2).  Then per-pixel:
      a = (p^2-q^2)/2 = 2*mu1*mu2        b = (p^2+q^2)/2 = mu1^2+mu2^2
      num = (a+C1)(c-a+C2)               den = (b+C1)(e-b+C2)
      dssim_mean = 0.5 - 0.5*mean(num/den)

Engine balancing (v2):
  - inputs are cast fp32->fp16 in the DMA itself (gpsimd SWDGE cast), so
    u,v tensor ops run in DVE 2x mode.
  - pass1 PSUM evacuation is split ACT/Pool (was: all ACT, the bottleneck).
  - ratio+mean fused in ONE custom DVE op (bitwise-not reciprocal seed +
    1 Newton step + multiply + accumulate), replacing reciprocal + TTR.
  - conv windows emit one matmul per contraction block (no accumulate/fresh
    segment splitting; per-element PSUM has_written handles mixed regions
    on hardware).  MERGED=False restores split segments for CoreSim.
"""

import math
import os

import numpy as np

import concourse.bass as bass
import concourse.bacc as bacc
import concourse.mybir as mybir
import concourse.tile as tile
import concourse.dve_ops as dve_ops
from concourse.dve_uop import DveOpSpec
from contextlib import ExitStack

F = 11
PAD = 5
C1 = 0.01 ** 2
C2 = 0.03 ** 2
H = 512
W = 512
NCORES = 8
BATCH = 32
CHAN = 3
B_PER_CORE = BATCH // NCORES          # 4
NCH = B_PER_CORE * CHAN               # 12 channel-images per core
NPOS = 4                              # 128-row blocks per image
TBW = 143                             # Toeplitz band tensor width

PSUM_DT = mybir.dt.float32   # matmul outputs must be fp32 (bass assert)
SBUF_DT = mybir.dt.float16
MERGED = os.environ.get("BASS_SSIM_MERGED", "1") == "1"   # 0 for CoreSim
DMA_CAST = os.environ.get("BASS_SSIM_DMACAST", "1") == "1"

f32 = mybir.dt.float32
f16 = mybir.dt.float16


# ---------------------------------------------------------------------------
# custom fused DVE ops
# ---------------------------------------------------------------------------
def _register_op(name, spec_fn):
    from concourse.dve_spec import lower, _has_src1 as has_src1

    op = dve_ops.DveOp(name, spec_fn(), subdim=False, uops_sha={})
    for ver in ("v3", "v4"):
        try:
            spec_obj = DveOpSpec(
                name=op.name, opcode=0, uops=lower(op.spec, ver=ver),
                rd1_en=has_src1(op.spec),
            )
            op.uops_sha[ver] = spec_obj.sha(ver)
        except Exception:
            pass
    if op.name not in dve_ops._SUB_OPCODE_FOR_NAME:
        dve_ops.OPS.append(op)
        dve_ops.CUSTOM_DVE_SPECS[op.name] = op.spec
        row = dve_ops._CUSTOM_DVE_ROW_BASE + len(dve_ops.OPS) - 1
        assert row < 0x20
        dve_ops._SUB_OPCODE_FOR_NAME[op.name] = row
    return op


def _numden_spec():
    # out = (Src0 + C0) * ((Src1 - Src0) + C1)
    from concourse.dve_spec import Spec, Src0, Src1, C0, C1 as SC1

    ref = lambda in0, in1, s0, s1, imm2: (
        (in0.astype(np.float32) + np.float32(s0))
        * ((in1.astype(np.float32) - in0.astype(np.float32)) + np.float32(s1))
    ).astype(np.float32)
    return Spec(body=(Src0 + C0) * ((Src1 - Src0) + SC1), reference=ref)


# seed scale / NR constant for the bitwise-not reciprocal approximation
# (same Chebyshev pair as RECIPROCAL_APPROX_FAST, one NR step: ~0.4% max err,
#  mean-of-ratios error far below the 2e-2 gate).
_RCP_S0 = -0.23549792
_RCP_S1 = 2.0017324


def _divacc_spec():
    # out = Src0 * recip1(Src1);  accum_out = sum(out)
    # recip1(d) = y0*(C1 - d*y0), y0 = bitcast(~d)*C0
    from concourse.dve_spec import (
        Spec, Src0, Src1, C0, C1 as SC1, Bin, AluOp, Zero)

    def ref(in0, in1, s0, s1, imm2):
        d = in1.astype(np.float32)
        not_d = (~d.view(np.int32)).view(np.float32)
        y0 = not_d * np.float32(s0)
        y1 = y0 * (np.float32(s1) - d * y0)
        out = (in0.astype(np.float32) * y1).astype(np.float32)
        return out, out.reshape(out.shape[0], -1).sum(axis=-1, keepdims=True)

    y0 = Bin(AluOp.BITWISE_NOT, Src1, Src1) * C0
    y1 = y0 * (SC1 - Src1 * y0)
    return Spec(body=Src0 * y1, accum=AluOp.ADD, accum_init=Zero,
                reference=ref)


NUMOP = _register_op("SSIM_NUMDEN_ANT", _numden_spec)
DIVACC = _register_op("SSIM_DIVACC_ANT", _divacc_spec)


# ---------------------------------------------------------------------------
# host-side kernel factorization
# ---------------------------------------------------------------------------
def _round_moment(k, nmom=2):
    """Round 1-D kernel factor to fp16 preserving moments (greedy)."""
    k = np.asarray(k, np.float64)
    n = len(k)
    kd = k.astype(np.float16).astype(np.float64)
    lo = np.minimum(np.nextafter(k.astype(np.float16), -np.inf).astype(np.float64), kd)
    hi = np.maximum(np.nextafter(k.astype(np.float16), np.inf).astype(np.float64), kd)
    cand = [np.array([lo[i], hi[i]]) for i in range(n)]
    idx = np.arange(n, dtype=np.float64) - (n - 1) / 2
    moms = np.stack([idx ** m for m in range(nmom + 1)])
    targ = moms @ k
    scale = np.abs(moms) @ np.abs(k) + 1e-300
    best = np.array([c[np.argmin(np.abs(c - k[i]))] for i, c in enumerate(cand)])

    def cost(v):
        return np.sum(((moms @ v - targ) / scale) ** 2)

    cur = cost(best)
    for _ in range(100):
        improved = False
        for i in range(n):
            for c in cand[i]:
                if c != best[i]:
                    t = best.copy()
                    t[i] = c
                    ct = cost(t)
                    if ct < cur - 1e-30:
                        best, cur = t, ct
                        improved = True
        if not improved:
            break
    return best


def _factor_channel_kernels(gauss_kernel):
    """[3,1,11,11] -> per-channel fp16 (kc, kr) rank-1 factors."""
    kcs, krs = [], []
    for ch in range(CHAN):
        k2d = np.asarray(gauss_kernel[ch, 0], np.float64)
        U, s, Vt = np.linalg.svd(k2d)
        kc = U[:, 0] * math.sqrt(s[0])
        kr = Vt[0] * math.sqrt(s[0])
        if kc.sum() < 0:
            kc, kr = -kc, -kr
        rec = np.abs(np.outer(kc, kr) - k2d).max()
        assert rec <= 1e-5 * max(1e-30, np.abs(k2d).max()), (
            f"gauss_kernel channel {ch} is not rank-1 (err {rec}); "
            "this kernel only supports separable filters"
        )
        kcs.append(_round_moment(kc))
        krs.append(_round_moment(kr))
    return kcs, krs


def _make_tfull(k1d):
    """T[k, j] = k1d[j - k] for j-k in [0,11); [128, TBW] float16."""
    T = np.zeros((128, TBW), np.float64)
    for d in range(F):
        j = np.arange(128) + d
        valid = j < TBW
        T[np.arange(128)[valid], j[valid]] = k1d[d]
    return T.astype(np.float16)


def _window(kb):
    lo = max(0, 128 * kb - PAD)
    hi = min(512, 128 * kb + 128 + PAD)
    return lo, hi, lo - 128 * kb + PAD


# ---------------------------------------------------------------------------
# device program
# ---------------------------------------------------------------------------
def build_program(nc: bass.Bass):
    im1 = nc.dram_tensor("im1", [B_PER_CORE, CHAN, H, W], f32, kind="ExternalInput").ap()
    im2 = nc.dram_tensor("im2", [B_PER_CORE, CHAN, H, W], f32, kind="ExternalInput").ap()
    # Toeplitz band tensors, [128, CHAN*TBW]: pass1 (kc), pass2 (kr), +-kr/2
    tb1 = nc.dram_tensor("tb1", [128, CHAN * TBW], f16, kind="ExternalInput").ap()
    tb2 = nc.dram_tensor("tb2", [128, CHAN * TBW], f16, kind="ExternalInput").ap()
    tb2h = nc.dram_tensor("tb2h", [128, CHAN * TBW], f16, kind="ExternalInput").ap()
    tb2hn = nc.dram_tensor("tb2hn", [128, CHAN * TBW], f16, kind="ExternalInput").ap()
    out = nc.dram_tensor("acc", [128, NCH], f32, kind="ExternalOutput").ap()

    with tile.TileContext(nc) as tc:
        with ExitStack() as ctx:
            _build_tile(ctx, tc, im1, im2, (tb1, tb2, tb2h, tb2hn), out)
    nc.compile()
    return nc


def _build_tile(ctx, tc, im1, im2, tbs, out):
    nc = tc.nc
    tb_pool = ctx.enter_context(tc.tile_pool(name="tb", bufs=1))
    in_pool = ctx.enter_context(tc.tile_pool(name="inp", bufs=2))
    sig_pool = ctx.enter_context(tc.tile_pool(name="sig", bufs=2))
    zt_pool = ctx.enter_context(tc.tile_pool(name="zt", bufs=2))
    p1_pool = ctx.enter_context(tc.tile_pool(name="p1", bufs=2, space="PSUM"))
    p2_pool = ctx.enter_context(tc.tile_pool(name="p2", bufs=4, space="PSUM"))
    fld_pool = ctx.enter_context(tc.tile_pool(name="fld", bufs=2))
    mth_pool = ctx.enter_context(tc.tile_pool(name="mth", bufs=2))
    acc_pool = ctx.enter_context(tc.tile_pool(name="accp", bufs=1))

    # band tensors in SBUF: [128, CHAN*TBW] each
    tbt = []
    for name, t in zip(("tb1", "tb2", "tb2h", "tb2hn"), tbs):
        st = tb_pool.tile([128, CHAN * TBW], f16, name=f"s_{name}")
        nc.sync.dma_start(st[:], t[:])
        tbt.append(st)
    stb1, stb2, stb2h, stb2hn = tbt

    acc = acc_pool.tile([128, NCH], f32, name="acc_sbuf")

    def conv_group(ps, ps_off, srcs, ch):
        """Windowed banded matmuls accumulating into ps[:, ps_off:ps_off+512].

        srcs: list of (stb_tile, lh_col_fn) pairs; lh_col_fn(kb) gives the
        lhsT AP for contraction block kb.  MERGED emits one matmul per
        window (mixed accumulate/fresh regions resolved by per-element
        PSUM has_written on HW); otherwise split into uniform segments
        for CoreSim.
        """
        plan = []
        for si, (stb, lh_fn) in enumerate(srcs):
            prev_hi = None
            for kb in range(4):
                lo, hi, off = _window(kb)
                if MERGED or si > 0 or prev_hi is None:
                    segs = [(lo, hi)]
                else:
                    segs = [(lo, prev_hi), (prev_hi, hi)]
                for s0, s1 in segs:
                    plan.append((lh_fn, kb, stb, s0, s1, off + (s0 - lo)))
                prev_hi = hi
        for i, (lh_fn, kb, stb, s0, s1, o) in enumerate(plan):
            nc.tensor.matmul(
                ps[:, ps_off + s0: ps_off + s1],
                lhsT=lh_fn(kb),
                rhs=stb[:, ch * TBW + o: ch * TBW + o + (s1 - s0)],
                start=(i == 0),
                stop=(i == len(plan) - 1),
            )

    for ci in range(NCH):
        img, ch = divmod(ci, CHAN)

        # ---- load x, y as fp16 [128, 4*512] (4 row-blocks side by side);
        #      the SWDGE (gpsimd) DMA casts fp32->fp16 in flight ----
        if DMA_CAST:
            x = in_pool.tile([128, 4 * W], f16, tag="x")
            y = in_pool.tile([128, 4 * W], f16, tag="y")
            nc.gpsimd.dma_start(x[:].rearrange("p (a w) -> p a w", a=4),
                                im1[img, ch].rearrange("(a p) w -> p a w", p=128))
            nc.gpsimd.dma_start(y[:].rearrange("p (a w) -> p a w", a=4),
                                im2[img, ch].rearrange("(a p) w -> p a w", p=128))
        else:
            x = in_pool.tile([128, 4 * W], f32, tag="x")
            y = in_pool.tile([128, 4 * W], f32, tag="y")
            nc.sync.dma_start(x[:].rearrange("p (a w) -> p a w", a=4),
                              im1[img, ch].rearrange("(a p) w -> p a w", p=128))
            nc.sync.dma_start(y[:].rearrange("p (a w) -> p a w", a=4),
                              im2[img, ch].rearrange("(a p) w -> p a w", p=128))

        # ---- u on Pool (SBUF-only TT; Pool cannot touch PSUM), v on DVE,
        #      u2, v2 on DVE (fp16 2x) ----
        u = sig_pool.tile([128, 4 * W], SBUF_DT, tag="u")
        v = sig_pool.tile([128, 4 * W], SBUF_DT, tag="v")
        nc.gpsimd.tensor_add(u[:], x[:], y[:])
        nc.vector.tensor_sub(v[:], x[:], y[:])
        u2 = sig_pool.tile([128, 4 * W], SBUF_DT, tag="u2")
        v2 = sig_pool.tile([128, 4 * W], SBUF_DT, tag="v2")
        nc.vector.tensor_tensor(u2[:], u[:], u[:], mybir.AluOpType.mult)
        nc.vector.tensor_tensor(v2[:], v[:], v[:], mybir.AluOpType.mult)

        # ---- pass 1: ZT_g[c, o] = sum_r g[r, c] * kc[o - r + 5] ----
        # Two column-blocks share one [128,1024] PSUM tile (one accumulation
        # group, halves the evacuation instruction count).  Evacuation is
        # ACT-heavy with one copy on DVE for balance.
        zts = []
        for gi, g in enumerate((u, v, u2, v2)):
            zt = zt_pool.tile([128, 4 * W], SBUF_DT, tag=f"zt{gi}")
            for cbp in range(2):
                ps = p1_pool.tile([128, 2 * W], PSUM_DT, tag="p1")
                srcs = []
                for half in range(2):
                    cb = 2 * cbp + half
                    lh = (lambda g_, cb_: lambda kb: g_[
                        :, kb * W + cb_ * 128: kb * W + cb_ * 128 + 128])(g, cb)
                    srcs.append((half * W, stb1, lh))
                plan = []
                for ps_off, stb, lh_fn in srcs:
                    prev_hi = None
                    for kb in range(4):
                        lo, hi, off = _window(kb)
                        if MERGED or prev_hi is None:
                            segs = [(lo, hi)]
                        else:
                            segs = [(lo, prev_hi), (prev_hi, hi)]
                        for s0, s1 in segs:
                            plan.append([ps_off, lh_fn, kb, stb, s0, s1,
                                         off + (s0 - lo), prev_hi is None,
                                         False])
                        prev_hi = hi
                    plan[-1][-1] = True  # stop at each half's last matmul
                for ps_off, lh_fn, kb, stb, s0, s1, o, first, last in plan:
                    nc.tensor.matmul(
                        ps[:, ps_off + s0: ps_off + s1],
                        lhsT=lh_fn(kb),
                        rhs=stb[:, ch * TBW + o: ch * TBW + o + (s1 - s0)],
                        start=first,
                        stop=last,
                    )
                dst = zt[:, cbp * 2 * W:(cbp + 1) * 2 * W]
                nc.scalar.activation(
                    dst, ps[:], mybir.ActivationFunctionType.Copy)
            zts.append(zt)
        ztu, ztv, ztu2, ztv2 = zts

        # ---- pass 2 + math, per position ----
        FIELD_SRC = {
            "p": ((ztu, stb2),),
            "q": ((ztv, stb2),),
            "c": ((ztu2, stb2h), (ztv2, stb2hn)),
            "e": ((ztu2, stb2h), (ztv2, stb2h)),
        }
        num = mth_pool.tile([128, 4 * W], f32, tag="num")
        den = mth_pool.tile([128, 4 * W], f32, tag="den")
        for m in range(NPOS):
            fld = {}
            for fname, srcs in FIELD_SRC.items():
                ps = p2_pool.tile([128, W], PSUM_DT, tag="p2")
                pairs = []
                for zt, stb in srcs:
                    lh = (lambda zt_, m_: lambda cb: zt_[
                        :, cb * W + m_ * 128: cb * W + m_ * 128 + 128])(zt, m)
                    pairs.append((stb, lh))
                conv_group(ps, 0, pairs, ch)
                fld[fname] = ps
            # squares with folded /2 (scale 1/sqrt2), evacuating p,q PSUM
            P2 = fld_pool.tile([128, W], SBUF_DT, tag="P2")
            Q2 = fld_pool.tile([128, W], SBUF_DT, tag="Q2")
            isq2 = 1.0 / math.sqrt(2.0)
            nc.scalar.activation(P2[:], fld["p"][:],
                                 mybir.ActivationFunctionType.Square, scale=isq2)
            nc.scalar.activation(Q2[:], fld["q"][:],
                                 mybir.ActivationFunctionType.Square, scale=isq2)
            a = mth_pool.tile([128, W], SBUF_DT, tag="a")
            b = mth_pool.tile([128, W], SBUF_DT, tag="b")
            nc.vector.tensor_sub(a[:], P2[:], Q2[:])
            nc.vector.tensor_add(b[:], P2[:], Q2[:])
            nc.vector._custom_dve(NUMOP, out=num[:, m * W:(m + 1) * W],
                                  in0=a[:], in1=fld["c"][:], s0=C1, s1=C2)
            nc.vector._custom_dve(NUMOP, out=den[:, m * W:(m + 1) * W],
                                  in0=b[:], in1=fld["e"][:], s0=C1, s1=C2)
        # fused ratio + reduce over the whole channel-image:
        # acc[ci] = sum(num * recip1(den))
        scr = mth_pool.tile([128, 4 * W], f32, tag="scr")
        nc.vector._custom_dve(
            DIVACC, out=scr[:], in0=num[:], in1=den[:],
            s0=_RCP_S0, s1=_RCP_S1,
            accum_out=acc[:, ci:ci + 1],
        )

    nc.sync.dma_start(out[:], acc[:])


# ---------------------------------------------------------------------------
# host entry point
# ---------------------------------------------------------------------------
_PROGRAM_CACHE = {}


def _get_program():
    if "nc" not in _PROGRAM_CACHE:
        nc = bacc.Bacc("TRN2", target_bir_lowering=False, debug=False,
                       num_devices=NCORES)
        _PROGRAM_CACHE["nc"] = build_program(nc)
    return _PROGRAM_CACHE["nc"]


def _host_tensors(gauss_kernel):
    kcs, krs = _factor_channel_kernels(np.asarray(gauss_kernel, np.float32))

    def pack(k1ds):
        # [128, CHAN*TBW]
        return np.concatenate([_make_tfull(k) for k in k1ds], axis=1)

    tb1 = pack(kcs)
    tb2 = pack(krs)
    tb2h = pack([0.5 * np.asarray(k, np.float64) for k in krs])
    tb2hn = pack([-0.5 * np.asarray(k, np.float64) for k in krs])
    return tb1, tb2, tb2h, tb2hn


def make_in_maps(image1, image2, gauss_kernel):
    image1 = np.asarray(image1, np.float32)
    image2 = np.asarray(image2, np.float32)
    tb1, tb2, tb2h, tb2hn = _host_tensors(gauss_kernel)
    in_maps = []
    for c in range(NCORES):
        s = slice(c * B_PER_CORE, (c + 1) * B_PER_CORE)
        in_maps.append({
            "im1": np.ascontiguousarray(image1[s]),
            "im2": np.ascontiguousarray(image2[s]),
            "tb1": tb1, "tb2": tb2, "tb2h": tb2h, "tb2hn": tb2hn,
        })
    return in_maps


def finish(acc_list):
    total = sum(np.asarray(a, np.float64).sum() for a in acc_list)
    n = float(BATCH * CHAN * H * W)
    return np.float32(0.5 - 0.5 * total / n)


def kernel(image1, image2, gauss_kernel):
    from concourse.bass_utils import run_bass_kernel_spmd

    nc = _get_program()
    in_maps = make_in_maps(image1, image2, gauss_kernel)
    res = run_bass_kernel_spmd(nc, in_maps, core_ids=list(range(NCORES)))
    return finish([r["acc"] for r in res.results])


# revision 12
# speedup vs baseline: 1.1312x; 1.0083x over previous
"""DSSIM loss on 8 Trainium2 NeuronCores (Bass/Tile).

Strategy (pure data parallel over batch, 4 images/core, 12 channel-images/core):
  signals u = x+y, v = x-y, u^2, v^2 are blurred with the separable 11x11
  kernel via two windowed "X-stationary" matmul passes:
      matmul(psum, lhsT=data_block[K=128,M=128], rhs=Toeplitz_band[K=128,N<=138])
  contracts the partition axis and emits the conv TRANSPOSED, so no explicit
  transpose is ever materialized; PSUM has_written accumulation handles the
  band-window overlaps and zero padding (clamped windows).
  Pass2 accumulates PE-side linear combos:  p=blur2(u), q=blur2(v),
  c=(blur2(u^2)-blur2(v^2))/2 = 2*blur2(xy),  e=(blur2(u^2)+blur2(v^2))/2
  = blur2(x^2+y^2).  Then per-pixel:
      a = (p^2-q^2)/2 = 2*mu1*mu2        b = (p^2+q^2)/2 = mu1^2+mu2^2
      num = (a+C1)(c-a+C2)               den = (b+C1)(e-b+C2)
      dssim_mean = 0.5 - 0.5*mean(num/den)

Engine balancing (v2):
  - inputs are cast fp32->fp16 in the DMA itself (gpsimd SWDGE cast), so
    u,v tensor ops run in DVE 2x mode.
  - pass1 PSUM evacuation is split ACT/Pool (was: all ACT, the bottleneck).
  - ratio+mean fused in ONE custom DVE op (bitwise-not reciprocal seed +
    1 Newton step + multiply + accumulate), replacing reciprocal + TTR.
  - conv windows emit one matmul per contraction block (no accumulate/fresh
    segment splitting; per-element PSUM has_written handles mixed regions
    on hardware).  MERGED=False restores split segments for CoreSim.
"""

import math
import os

import numpy as np

import concourse.bass as bass
import concourse.bacc as bacc
import concourse.mybir as mybir
import concourse.tile as tile
import concourse.dve_ops as dve_ops
from concourse.dve_uop import DveOpSpec
from contextlib import ExitStack

F = 11
PAD = 5
C1 = 0.01 ** 2
C2 = 0.03 ** 2
H = 512
W = 512
NCORES = 8
BATCH = 32
CHAN = 3
B_PER_CORE = BATCH // NCORES          # 4
NCH = B_PER_CORE * CHAN               # 12 channel-images per core
NPOS = 4                              # 128-row blocks per image
TBW = 143                             # Toeplitz band tensor width

PSUM_DT = mybir.dt.float32   # matmul outputs must be fp32 (bass assert)
SBUF_DT = mybir.dt.float16
MERGED = os.environ.get("BASS_SSIM_MERGED", "1") == "1"   # 0 for CoreSim
DMA_CAST = os.environ.get("BASS_SSIM_DMACAST", "1") == "1"

f32 = mybir.dt.float32
f16 = mybir.dt.float16


# ---------------------------------------------------------------------------
# custom fused DVE ops
# ---------------------------------------------------------------------------
def _register_op(name, spec_fn):
    from concourse.dve_spec import lower, _has_src1 as has_src1

    op = dve_ops.DveOp(name, spec_fn(), subdim=False, uops_sha={})
    for ver in ("v3", "v4"):
        try:
            spec_obj = DveOpSpec(
                name=op.name, opcode=0, uops=lower(op.spec, ver=ver),
                rd1_en=has_src1(op.spec),
            )
            op.uops_sha[ver] = spec_obj.sha(ver)
        except Exception:
            pass
    if op.name not in dve_ops._SUB_OPCODE_FOR_NAME:
        dve_ops.OPS.append(op)
        dve_ops.CUSTOM_DVE_SPECS[op.name] = op.spec
        row = dve_ops._CUSTOM_DVE_ROW_BASE + len(dve_ops.OPS) - 1
        assert row < 0x20
        dve_ops._SUB_OPCODE_FOR_NAME[op.name] = row
    return op


def _numden_spec():
    # out = (Src0 + C0) * ((Src1 - Src0) + C1)
    from concourse.dve_spec import Spec, Src0, Src1, C0, C1 as SC1

    ref = lambda in0, in1, s0, s1, imm2: (
        (in0.astype(np.float32) + np.float32(s0))
        * ((in1.astype(np.float32) - in0.astype(np.float32)) + np.float32(s1))
    ).astype(np.float32)
    return Spec(body=(Src0 + C0) * ((Src1 - Src0) + SC1), reference=ref)


# seed scale / NR constant for the bitwise-not reciprocal approximation
# (same Chebyshev pair as RECIPROCAL_APPROX_FAST, one NR step: ~0.4% max err,
#  mean-of-ratios error far below the 2e-2 gate).
_RCP_S0 = -0.23549792
_RCP_S1 = 2.0017324


def _divacc_spec():
    # out = Src0 * recip1(Src1);  accum_out = sum(out)
    # recip1(d) = y0*(C1 - d*y0), y0 = bitcast(~d)*C0
    from concourse.dve_spec import (
        Spec, Src0, Src1, C0, C1 as SC1, Bin, AluOp, Zero)

    def ref(in0, in1, s0, s1, imm2):
        d = in1.astype(np.float32)
        not_d = (~d.view(np.int32)).view(np.float32)
        y0 = not_d * np.float32(s0)
        y1 = y0 * (np.float32(s1) - d * y0)
        out = (in0.astype(np.float32) * y1).astype(np.float32)
        return out, out.reshape(out.shape[0], -1).sum(axis=-1, keepdims=True)

    y0 = Bin(AluOp.BITWISE_NOT, Src1, Src1) * C0
    y1 = y0 * (SC1 - Src1 * y0)
    return Spec(body=Src0 * y1, accum=AluOp.ADD, accum_init=Zero,
                reference=ref)


NUMOP = _register_op("SSIM_NUMDEN_ANT", _numden_spec)
DIVACC = _register_op("SSIM_DIVACC_ANT", _divacc_spec)


# ---------------------------------------------------------------------------
# host-side kernel factorization
# ---------------------------------------------------------------------------
def _round_moment(k, nmom=2):
    """Round 1-D kernel factor to fp16 preserving moments (greedy)."""
    k = np.asarray(k, np.float64)
    n = len(k)
    kd = k.astype(np.float16).astype(np.float64)
    lo = np.minimum(np.nextafter(k.astype(np.float16), -np.inf).astype(np.float64), kd)
    hi = np.maximum(np.nextafter(k.astype(np.float16), np.inf).astype(np.float64), kd)
    cand = [np.array([lo[i], hi[i]]) for i in range(n)]
    idx = np.arange(n, dtype=np.float64) - (n - 1) / 2
    moms = np.stack([idx ** m for m in range(nmom + 1)])
    targ = moms @ k
    scale = np.abs(moms) @ np.abs(k) + 1e-300
    best = np.array([c[np.argmin(np.abs(c - k[i]))] for i, c in enumerate(cand)])

    def cost(v):
        return np.sum(((moms @ v - targ) / scale) ** 2)

    cur = cost(best)
    for _ in range(100):
        improved = False
        for i in range(n):
            for c in cand[i]:
                if c != best[i]:
                    t = best.copy()
                    t[i] = c
                    ct = cost(t)
                    if ct < cur - 1e-30:
                        best, cur = t, ct
                        improved = True
        if not improved:
            break
    return best


def _factor_channel_kernels(gauss_kernel):
    """[3,1,11,11] -> per-channel fp16 (kc, kr) rank-1 factors."""
    kcs, krs = [], []
    for ch in range(CHAN):
        k2d = np.asarray(gauss_kernel[ch, 0], np.float64)
        U, s, Vt = np.linalg.svd(k2d)
        kc = U[:, 0] * math.sqrt(s[0])
        kr = Vt[0] * math.sqrt(s[0])
        if kc.sum() < 0:
            kc, kr = -kc, -kr
        rec = np.abs(np.outer(kc, kr) - k2d).max()
        assert rec <= 1e-5 * max(1e-30, np.abs(k2d).max()), (
            f"gauss_kernel channel {ch} is not rank-1 (err {rec}); "
            "this kernel only supports separable filters"
        )
        kcs.append(_round_moment(kc))
        krs.append(_round_moment(kr))
    return kcs, krs


def _make_tfull(k1d):
    """T[k, j] = k1d[j - k] for j-k in [0,11); [128, TBW] float16."""
    T = np.zeros((128, TBW), np.float64)
    for d in range(F):
        j = np.arange(128) + d
        valid = j < TBW
        T[np.arange(128)[valid], j[valid]] = k1d[d]
    return T.astype(np.float16)


def _window(kb):
    lo = max(0, 128 * kb - PAD)
    hi = min(512, 128 * kb + 128 + PAD)
    return lo, hi, lo - 128 * kb + PAD


# ---------------------------------------------------------------------------
# device program
# ---------------------------------------------------------------------------
def build_program(nc: bass.Bass):
    im1 = nc.dram_tensor("im1", [B_PER_CORE, CHAN, H, W], f32, kind="ExternalInput").ap()
    im2 = nc.dram_tensor("im2", [B_PER_CORE, CHAN, H, W], f32, kind="ExternalInput").ap()
    # Toeplitz band tensors, [128, CHAN*TBW]: pass1 (kc), pass2 (kr), +-kr/2
    tb1 = nc.dram_tensor("tb1", [128, CHAN * TBW], f16, kind="ExternalInput").ap()
    tb2 = nc.dram_tensor("tb2", [128, CHAN * TBW], f16, kind="ExternalInput").ap()
    tb2h = nc.dram_tensor("tb2h", [128, CHAN * TBW], f16, kind="ExternalInput").ap()
    tb2hn = nc.dram_tensor("tb2hn", [128, CHAN * TBW], f16, kind="ExternalInput").ap()
    out = nc.dram_tensor("acc", [128, NCH], f32, kind="ExternalOutput").ap()

    with tile.TileContext(nc) as tc:
        with ExitStack() as ctx:
            _build_tile(ctx, tc, im1, im2, (tb1, tb2, tb2h, tb2hn), out)
    nc.compile()
    return nc


def _build_tile(ctx, tc, im1, im2, tbs, out):
    nc = tc.nc
    tb_pool = ctx.enter_context(tc.tile_pool(name="tb", bufs=1))
    in_pool = ctx.enter_context(tc.tile_pool(name="inp", bufs=3))
    sig_pool = ctx.enter_context(tc.tile_pool(name="sig", bufs=3))
    zt_pool = ctx.enter_context(tc.tile_pool(name="zt", bufs=3))
    p1_pool = ctx.enter_context(tc.tile_pool(name="p1", bufs=2, space="PSUM"))
    p2_pool = ctx.enter_context(tc.tile_pool(name="p2", bufs=4, space="PSUM"))
    fld_pool = ctx.enter_context(tc.tile_pool(name="fld", bufs=4))
    mth_pool = ctx.enter_context(tc.tile_pool(name="mth", bufs=3))
    scr_pool = ctx.enter_context(tc.tile_pool(name="scr", bufs=1))
    acc_pool = ctx.enter_context(tc.tile_pool(name="accp", bufs=1))

    # band tensors in SBUF: [128, CHAN*TBW] each
    tbt = []
    for name, t in zip(("tb1", "tb2", "tb2h", "tb2hn"), tbs):
        st = tb_pool.tile([128, CHAN * TBW], f16, name=f"s_{name}")
        nc.sync.dma_start(st[:], t[:])
        tbt.append(st)
    stb1, stb2, stb2h, stb2hn = tbt

    acc = acc_pool.tile([128, NCH], f32, name="acc_sbuf")

    def conv_group(ps, ps_off, srcs, ch):
        """Windowed banded matmuls accumulating into ps[:, ps_off:ps_off+512].

        srcs: list of (stb_tile, lh_col_fn) pairs; lh_col_fn(kb) gives the
        lhsT AP for contraction block kb.  MERGED emits one matmul per
        window (mixed accumulate/fresh regions resolved by per-element
        PSUM has_written on HW); otherwise split into uniform segments
        for CoreSim.
        """
        plan = []
        for si, (stb, lh_fn) in enumerate(srcs):
            prev_hi = None
            for kb in range(4):
                lo, hi, off = _window(kb)
                if MERGED or si > 0 or prev_hi is None:
                    segs = [(lo, hi)]
                else:
                    segs = [(lo, prev_hi), (prev_hi, hi)]
                for s0, s1 in segs:
                    plan.append((lh_fn, kb, stb, s0, s1, off + (s0 - lo)))
                prev_hi = hi
        for i, (lh_fn, kb, stb, s0, s1, o) in enumerate(plan):
            nc.tensor.matmul(
                ps[:, ps_off + s0: ps_off + s1],
                lhsT=lh_fn(kb),
                rhs=stb[:, ch * TBW + o: ch * TBW + o + (s1 - s0)],
                start=(i == 0),
                stop=(i == len(plan) - 1),
            )

    for ci in range(NCH):
        img, ch = divmod(ci, CHAN)

        # ---- load x, y as fp16 [128, 4*512] (4 row-blocks side by side);
        #      the SWDGE (gpsimd) DMA casts fp32->fp16 in flight ----
        if DMA_CAST:
            x = in_pool.tile([128, 4 * W], f16, tag="x")
            y = in_pool.tile([128, 4 * W], f16, tag="y")
            nc.gpsimd.dma_start(x[:].rearrange("p (a w) -> p a w", a=4),
                                im1[img, ch].rearrange("(a p) w -> p a w", p=128))
            nc.gpsimd.dma_start(y[:].rearrange("p (a w) -> p a w", a=4),
                                im2[img, ch].rearrange("(a p) w -> p a w", p=128))
        else:
            x = in_pool.tile([128, 4 * W], f32, tag="x")
            y = in_pool.tile([128, 4 * W], f32, tag="y")
            nc.sync.dma_start(x[:].rearrange("p (a w) -> p a w", a=4),
                              im1[img, ch].rearrange("(a p) w -> p a w", p=128))
            nc.sync.dma_start(y[:].rearrange("p (a w) -> p a w", a=4),
                              im2[img, ch].rearrange("(a p) w -> p a w", p=128))

        # ---- u on Pool (SBUF-only TT; Pool cannot touch PSUM), v on DVE,
        #      u2, v2 on DVE (fp16 2x) ----
        u = sig_pool.tile([128, 4 * W], SBUF_DT, tag="u")
        v = sig_pool.tile([128, 4 * W], SBUF_DT, tag="v")
        nc.gpsimd.tensor_add(u[:], x[:], y[:])
        nc.vector.tensor_sub(v[:], x[:], y[:])
        u2 = sig_pool.tile([128, 4 * W], SBUF_DT, tag="u2")
        v2 = sig_pool.tile([128, 4 * W], SBUF_DT, tag="v2")
        nc.vector.tensor_tensor(u2[:], u[:], u[:], mybir.AluOpType.mult)
        nc.vector.tensor_tensor(v2[:], v[:], v[:], mybir.AluOpType.mult)

        # ---- pass 1: ZT_g[c, o] = sum_r g[r, c] * kc[o - r + 5] ----
        # Two column-blocks share one [128,1024] PSUM tile (one accumulation
        # group, halves the evacuation instruction count).  Evacuation is
        # ACT-heavy with one copy on DVE for balance.
        zts = []
        for gi, g in enumerate((u, v, u2, v2)):
            zt = zt_pool.tile([128, 4 * W], SBUF_DT, tag=f"zt{gi}")
            for cbp in range(2):
                ps = p1_pool.tile([128, 2 * W], PSUM_DT, tag="p1")
                srcs = []
                for half in range(2):
                    cb = 2 * cbp + half
                    lh = (lambda g_, cb_: lambda kb: g_[
                        :, kb * W + cb_ * 128: kb * W + cb_ * 128 + 128])(g, cb)
                    srcs.append((half * W, stb1, lh))
                plan = []
                for ps_off, stb, lh_fn in srcs:
                    prev_hi = None
                    for kb in range(4):
                        lo, hi, off = _window(kb)
                        if MERGED or prev_hi is None:
                            segs = [(lo, hi)]
                        else:
                            segs = [(lo, prev_hi), (prev_hi, hi)]
                        for s0, s1 in segs:
                            plan.append([ps_off, lh_fn, kb, stb, s0, s1,
                                         off + (s0 - lo), prev_hi is None,
                                         False])
                        prev_hi = hi
                    plan[-1][-1] = True  # stop at each half's last matmul
                for ps_off, lh_fn, kb, stb, s0, s1, o, first, last in plan:
                    nc.tensor.matmul(
                        ps[:, ps_off + s0: ps_off + s1],
                        lhsT=lh_fn(kb),
                        rhs=stb[:, ch * TBW + o: ch * TBW + o + (s1 - s0)],
                        start=first,
                        stop=last,
                    )
                dst = zt[:, cbp * 2 * W:(cbp + 1) * 2 * W]
                nc.scalar.activation(
                    dst, ps[:], mybir.ActivationFunctionType.Copy)
            zts.append(zt)
        ztu, ztv, ztu2, ztv2 = zts

        # ---- pass 2 + math, per position ----
        FIELD_SRC = {
            "p": ((ztu, stb2),),
            "q": ((ztv, stb2),),
            "c": ((ztu2, stb2h), (ztv2, stb2hn)),
            "e": ((ztu2, stb2h), (ztv2, stb2h)),
        }
        num = mth_pool.tile([128, 4 * W], SBUF_DT, tag="num")
        den = mth_pool.tile([128, 4 * W], f32, tag="den")
        for m in range(NPOS):
            fld = {}
            for fname, srcs in FIELD_SRC.items():
                ps = p2_pool.tile([128, W], PSUM_DT, tag="p2")
                pairs = []
                for zt, stb in srcs:
                    lh = (lambda zt_, m_: lambda cb: zt_[
                        :, cb * W + m_ * 128: cb * W + m_ * 128 + 128])(zt, m)
                    pairs.append((stb, lh))
                conv_group(ps, 0, pairs, ch)
                fld[fname] = ps
            # squares with folded /2 (scale 1/sqrt2), evacuating p,q PSUM
            P2 = fld_pool.tile([128, W], SBUF_DT, tag="P2")
            Q2 = fld_pool.tile([128, W], SBUF_DT, tag="Q2")
            isq2 = 1.0 / math.sqrt(2.0)
            nc.scalar.activation(P2[:], fld["p"][:],
                                 mybir.ActivationFunctionType.Square, scale=isq2)
            nc.scalar.activation(Q2[:], fld["q"][:],
                                 mybir.ActivationFunctionType.Square, scale=isq2)
            a = mth_pool.tile([128, W], SBUF_DT, tag="a")
            b = mth_pool.tile([128, W], SBUF_DT, tag="b")
            nc.vector.tensor_sub(a[:], P2[:], Q2[:])
            nc.vector.tensor_add(b[:], P2[:], Q2[:])
            nc.vector._custom_dve(NUMOP, out=num[:, m * W:(m + 1) * W],
                                  in0=a[:], in1=fld["c"][:], s0=C1, s1=C2)
            nc.vector._custom_dve(NUMOP, out=den[:, m * W:(m + 1) * W],
                                  in0=b[:], in1=fld["e"][:], s0=C1, s1=C2)
        # fused ratio + reduce over the whole channel-image:
        # acc[ci] = sum(num * recip1(den))
        scr = scr_pool.tile([128, 4 * W], SBUF_DT, tag="scr")
        nc.vector._custom_dve(
            DIVACC, out=scr[:], in0=num[:], in1=den[:],
            s0=_RCP_S0, s1=_RCP_S1,
            accum_out=acc[:, ci:ci + 1],
        )

    nc.sync.dma_start(out[:], acc[:])


# ---------------------------------------------------------------------------
# host entry point
# ---------------------------------------------------------------------------
_PROGRAM_CACHE = {}


def _get_program():
    if "nc" not in _PROGRAM_CACHE:
        nc = bacc.Bacc("TRN2", target_bir_lowering=False, debug=False,
                       num_devices=NCORES)
        _PROGRAM_CACHE["nc"] = build_program(nc)
    return _PROGRAM_CACHE["nc"]


def _host_tensors(gauss_kernel):
    kcs, krs = _factor_channel_kernels(np.asarray(gauss_kernel, np.float32))

    def pack(k1ds):
        # [128, CHAN*TBW]
        return np.concatenate([_make_tfull(k) for k in k1ds], axis=1)

    tb1 = pack(kcs)
    tb2 = pack(krs)
    tb2h = pack([0.5 * np.asarray(k, np.float64) for k in krs])
    tb2hn = pack([-0.5 * np.asarray(k, np.float64) for k in krs])
    return tb1, tb2, tb2h, tb2hn


def make_in_maps(image1, image2, gauss_kernel):
    image1 = np.asarray(image1, np.float32)
    image2 = np.asarray(image2, np.float32)
    tb1, tb2, tb2h, tb2hn = _host_tensors(gauss_kernel)
    in_maps = []
    for c in range(NCORES):
        s = slice(c * B_PER_CORE, (c + 1) * B_PER_CORE)
        in_maps.append({
            "im1": np.ascontiguousarray(image1[s]),
            "im2": np.ascontiguousarray(image2[s]),
            "tb1": tb1, "tb2": tb2, "tb2h": tb2h, "tb2hn": tb2hn,
        })
    return in_maps


def finish(acc_list):
    total = sum(np.asarray(a, np.float64).sum() for a in acc_list)
    n = float(BATCH * CHAN * H * W)
    return np.float32(0.5 - 0.5 * total / n)


def kernel(image1, image2, gauss_kernel):
    from concourse.bass_utils import run_bass_kernel_spmd

    nc = _get_program()
    in_maps = make_in_maps(image1, image2, gauss_kernel)
    res = run_bass_kernel_spmd(nc, in_maps, core_ids=list(range(NCORES)))
    return finish([r["acc"] for r in res.results])
